# revision 1
# baseline (speedup 1.0000x reference)
"""MoE layer (8 experts, top-2) on 8 Trainium2 NeuronCores, expert-parallel.

Strategy:
  - Host computes the (tiny) gating linear + softmax + top-2 routing,
    mirroring the reference ops so expert selection matches exactly.
  - Tokens are dispatched to their experts on the host (the all-to-all),
    one expert per NeuronCore. Each core runs the 1024->4096->1024 gelu
    FFN for its expert over its routed tokens (padded to a common
    capacity), with all matmuls in float32r at full PE rate.
  - Host applies combine weights and scatter-adds back to token order.

Device layout: activations are kept transposed ([feature, token]) so both
matmuls consume the natural [K, M] weight layout and the phase-1 output
feeds phase-2 without any on-chip transpose. The 4096-wide hidden dim is
processed in quarters: phase 1 produces a quarter of the hidden
activations for ALL tokens (SBUF-resident), phase 2 immediately folds it
into an SBUF-resident partial sum of y. Expert weights therefore stream
from HBM exactly once, which keeps the kernel tensor-engine-bound
(streaming them per token-group was HBM-bound under 8-core contention).
"""

import numpy as np

N_EMBED = 1024
FFN_HIDDEN = 4096
NUM_EXPERTS = 8
TOP_K = 2
P = 128
KT1 = N_EMBED // P       # 8  k-tiles, phase 1
MT1 = FFN_HIDDEN // P    # 32 m-tiles, phase 1
KT2 = FFN_HIDDEN // P    # 32 k-tiles, phase 2
MT2 = N_EMBED // P       # 8  m-tiles, phase 2
QF = 4                   # FFN hidden dim is processed in QF f-quarters so the
                         # resident hT slab stays small and weights stream once

LAST_RESULT = None       # BassKernelResults of the most recent run (debug/profiling)


def _ensure_axon_hooks():
    """Make `antenv.axon_hooks` importable so BASS_TRACE=1 degrades
    gracefully instead of crashing when the image lacks the module."""
    try:
        import antenv.axon_hooks  # noqa: F401
        return
    except ImportError:
        pass
    import sys
    import types

    m = types.ModuleType("antenv.axon_hooks")
    m._hook = None
    m.set_axon_ntff_profile_hook = lambda h: setattr(m, "_hook", h)
    m.get_axon_ntff_profile_hook = lambda: m._hook
    sys.modules["antenv.axon_hooks"] = m
    try:
        from trn_agent_boot.trn_boot import _ntff_profile_via_ctypes

        m.set_axon_ntff_profile_hook(_ntff_profile_via_ctypes("/opt/axon/libaxon_pjrt.so"))
    except Exception:
        pass


def _route(x2d, Wg, bg):
    """Top-2 gating. Mirrors the reference (jax softmax + lax.top_k) so the
    selected experts match it exactly; numpy fallback is numerically
    equivalent up to fp32 rounding."""
    try:
        import jax
        import jax.numpy as jnp

        gate = jax.nn.softmax(jnp.asarray(x2d) @ jnp.asarray(Wg) + jnp.asarray(bg), axis=-1)
        scores, idx = jax.lax.top_k(gate, TOP_K)
        scores = np.asarray(scores, dtype=np.float32)
        idx = np.asarray(idx)
    except Exception:
        logits = x2d @ Wg + bg
        m = logits.max(-1, keepdims=True)
        e = np.exp(logits - m)
        p = e / e.sum(-1, keepdims=True)
        order = np.argsort(-p, axis=-1, kind="stable")
        idx = order[:, :TOP_K]
        scores = np.take_along_axis(p, idx, axis=-1)
    scores = scores / (scores.sum(-1, keepdims=True) + 1e-8)
    return idx.astype(np.int64), scores.astype(np.float32)


def _chunks(width):
    """Split the token capacity (>=256) into matmul free-dim chunks, each
    in [256, 512] so every fp32r matmul runs at full PE rate. Smallest
    chunk first: it gates the very first matmul of the kernel."""
    out, rem = [], width
    while rem > 0:
        if rem <= 512:
            c = rem
        elif rem < 768:
            c = rem - 256
        else:
            c = 512
        out.append(c)
        rem -= c
    out.sort()
    res, off = [], 0
    for c in out:
        res.append((off, c))
        off += c
    return res




def _build_device_program(cap, p2_bf16=False):
    import concourse.tile as tile
    from concourse import bacc, mybir
    from concourse.tile_rust import add_dep_helper

    f32 = mybir.dt.float32
    f32r = mybir.dt.float32r
    p2dt = mybir.dt.bfloat16 if p2_bf16 else f32r
    gelu = mybir.ActivationFunctionType.Gelu_apprx_tanh
    ident = mybir.ActivationFunctionType.Identity

    nc = bacc.Bacc("TRN2", target_bir_lowering=False, debug=False,
                   num_devices=NUM_EXPERTS)

    MQ1 = MT1 // QF  # phase-1 m-tiles (f-tiles) per quarter
    KQ2 = KT2 // QF  # phase-2 k-tiles (f-tiles) per quarter

    xg_d = nc.dram_tensor("xg", [KT1, P, cap], f32r, kind="ExternalInput").ap()
    w1_d = nc.dram_tensor("w1t", [MT1, P, KT1 * P], f32r, kind="ExternalInput").ap()
    # w2 is laid out quarter-sliced: tile (q*MT2 + m) holds the KQ2 k-slices
    # of f-quarter q for output tile m
    w2_d = nc.dram_tensor("w2t", [QF * MT2, P, KQ2 * P], p2dt,
                          kind="ExternalInput").ap()
    b1_d = nc.dram_tensor("b1m", [P, MT1], f32, kind="ExternalInput").ap()
    b2_d = nc.dram_tensor("b2m", [P, MT2], f32, kind="ExternalInput").ap()
    y_d = nc.dram_tensor("yT", [MT2, P, cap], f32, kind="ExternalOutput").ap()

    chunks = _chunks(cap)

    with tile.TileContext(nc) as tc:
        with (
            tc.tile_pool(name="const", bufs=1) as const,
            tc.tile_pool(name="xp", bufs=1) as xp,
            tc.tile_pool(name="hp", bufs=1) as hp,
            tc.tile_pool(name="yp", bufs=1) as yp,
            tc.tile_pool(name="w1p", bufs=4) as w1p,
            tc.tile_pool(name="w2p", bufs=4) as w2p,
            tc.tile_pool(name="psp", bufs=8, space="PSUM") as psp,
            tc.tile_pool(name="op", bufs=6) as op,
        ):
            # one tile per k-tile of x, split at the first chunk boundary;
            # all first-chunk slices are issued before anything else (DMA
            # issue on the sequencer costs ~650ns each) so the first
            # k-accumulation starts as soon as possible
            first_cw = chunks[0][1]
            xg_sbs = []
            for kt in range(KT1):
                xt = xp.tile([P, cap], f32r, name=f"xg{kt}")
                nc.sync.dma_start(xt[:, :first_cw], xg_d[kt, :, :first_cw])
                xg_sbs.append(xt)
            b1_sb = const.tile([P, MT1], f32)
            nc.sync.dma_start(b1_sb[:], b1_d[:, :])
            b2_sb = const.tile([P, MT2], f32)
            nc.sync.dma_start(b2_sb[:], b2_d[:, :])
            for kt in range(KT1):
                nc.sync.dma_start(xg_sbs[kt][:, first_cw:], xg_d[kt, :, first_cw:])

            hT_sb = hp.tile([P, MQ1 * cap], p2dt)
            y_sb = yp.tile([P, MT2 * cap], f32)

            anchor_act = None
            for q in range(QF):
                # phase 1 (quarter q): hT = gelu(W1[:, fq].T @ xT + b1[fq])
                for mq in range(MQ1):
                    m = q * MQ1 + mq
                    w1m = w1p.tile([P, KT1 * P], f32r, tag="w1")
                    nc.gpsimd.dma_start(w1m[:], w1_d[m, :, :])
                    for ci, (cs, cw) in enumerate(chunks):
                        ps = psp.tile([P, cw], f32, tag="ps", name=f"ps{ci}")
                        for kt in range(KT1):
                            nc.tensor.matmul(
                                ps[:],
                                w1m[:, kt * P:(kt + 1) * P],
                                xg_sbs[kt][:, cs:cs + cw],
                                start=(kt == 0),
                                stop=(kt == KT1 - 1),
                            )
                        act = nc.scalar.activation(
                            hT_sb[:, mq * cap + cs:mq * cap + cs + cw],
                            ps[:],
                            gelu,
                            bias=b1_sb[:, m:m + 1],
                        )
                        if q == 0 and mq == 6 and ci == 0:
                            anchor_act = act.ins
                # phase 2 (quarter q): y (+)= W2[fq].T @ hT  [+ b2 on q=0]
                for m in range(MT2):
                    w2m = w2p.tile([P, KQ2 * P], p2dt, tag="w2")
                    w2dma = nc.gpsimd.dma_start(w2m[:], w2_d[q * MT2 + m, :, :])
                    if q == 0 and m < 2 and anchor_act is not None:
                        # keep w2 prefetches out of the prologue DMA queues;
                        # they are only needed once phase 1 is well underway
                        add_dep_helper(w2dma.ins, anchor_act, sync=False,
                                       reason="delay w2 prefetch past early phase-1")
                    # on the very last output tile, finish with the smallest
                    # chunk: its evacuate+store is the kernel's tail
                    mchunks = chunks
                    if q == QF - 1 and m == MT2 - 1:
                        mchunks = sorted(chunks, key=lambda c: -c[1])
                    for ci, (cs, cw) in enumerate(mchunks):
                        ps = psp.tile([P, cw], f32, tag="ps", name=f"ps{ci}")
                        for kq in range(KQ2):
                            nc.tensor.matmul(
                                ps[:],
                                w2m[:, kq * P:(kq + 1) * P],
                                hT_sb[:, kq * cap + cs:kq * cap + cs + cw],
                                start=(kq == 0),
                                stop=(kq == KQ2 - 1),
                            )
                        ysl = y_sb[:, m * cap + cs:m * cap + cs + cw]
                        if q == 0:
                            nc.scalar.activation(ysl, ps[:], ident,
                                                 bias=b2_sb[:, m:m + 1])
                        elif q < QF - 1:
                            nc.vector.tensor_add(ysl, ps[:], ysl)
                        else:
                            ot = op.tile([P, cw], f32, tag="o", name=f"o{ci}")
                            nc.vector.tensor_add(ot[:], ps[:], ysl)
                            nc.sync.dma_start(y_d[m, :, cs:cs + cw], ot[:])

    nc.compile()
    return nc


def kernel(x, Wg, bg, W1, b1, W2, b2):
    global LAST_RESULT
    _ensure_axon_hooks()
    from concourse.bass_utils import run_bass_kernel_spmd

    x = np.ascontiguousarray(np.asarray(x, dtype=np.float32))
    Wg = np.asarray(Wg, dtype=np.float32)
    bg = np.asarray(bg, dtype=np.float32)
    W1 = np.asarray(W1, dtype=np.float32)
    b1 = np.asarray(b1, dtype=np.float32)
    W2 = np.asarray(W2, dtype=np.float32)
    b2 = np.asarray(b2, dtype=np.float32)

    B, S, D = x.shape
    T = B * S
    xf = x.reshape(T, D)

    top_idx, top_w = _route(xf, Wg, bg)

    tok_idx = []
    tok_w = []
    for e in range(NUM_EXPERTS):
        sel = top_idx == e                       # [T, K]
        rows = np.nonzero(sel.any(axis=1))[0]
        tok_idx.append(rows)
        tok_w.append((top_w * sel).sum(axis=1)[rows].astype(np.float32))

    maxc = max(len(r) for r in tok_idx)
    cap = max(256, -(-maxc // 16) * 16)  # 64B-aligned rows, minimal padding

    import os as _os
    p2_bf16 = bool(_os.environ.get("MOE_P2_BF16"))
    nc = _build_device_program(cap, p2_bf16)

    in_maps = []
    for e in range(NUM_EXPERTS):
        idx_pad = np.zeros(cap, dtype=np.int64)
        idx_pad[:len(tok_idx[e])] = tok_idx[e]
        xg = np.ascontiguousarray(xf[idx_pad].T).reshape(KT1, P, cap)
        w1t = np.ascontiguousarray(
            W1[e].reshape(KT1, P, MT1, P).transpose(2, 1, 0, 3)
        ).reshape(MT1, P, KT1 * P)
        w2t = np.ascontiguousarray(
            W2[e].reshape(QF, KT2 // QF, P, MT2, P).transpose(0, 3, 2, 1, 4)
        ).reshape(QF * MT2, P, (KT2 // QF) * P)
        if p2_bf16:
            import ml_dtypes
            w2t = w2t.astype(ml_dtypes.bfloat16)
        in_maps.append({
            "xg": xg,
            "w1t": w1t,
            "w2t": w2t,
            "b1m": np.ascontiguousarray(b1[e].reshape(MT1, P).T),
            "b2m": np.ascontiguousarray(b2[e].reshape(MT2, P).T),
        })

    import os
    trace_cores = None
    if os.environ.get("MOE_TRACE_ALL"):
        trace_cores = list(range(NUM_EXPERTS))
    res = run_bass_kernel_spmd(nc, in_maps, core_ids=list(range(NUM_EXPERTS)),
                               trace_cores=trace_cores)
    LAST_RESULT = res

    out = np.zeros((T, D), dtype=np.float32)
    for e in range(NUM_EXPERTS):
        n_e = len(tok_idx[e])
        if n_e == 0:
            continue
        yT = res.results[e]["yT"].reshape(D, cap)
        out[tok_idx[e]] += tok_w[e][:, None] * yT[:, :n_e].T
    return out.reshape(B, S, D)



# revision 2
# speedup vs baseline: 1.1117x; 1.1117x over previous
"""MoE layer (8 experts, top-2) on 8 Trainium2 NeuronCores.

Strategy (v2): hidden-dim-split expert parallelism in bf16.
  - Host computes gating + top-2 routing (mirrors the reference ops).
  - Experts are sorted by token load and paired hot-with-cold; pair p is
    assigned to cores (2p, 2p+1), each core computing one HALF of the FFN
    hidden dim (2048 of 4096) for BOTH experts of the pair. This balances
    the per-core matmul work to ~(L_hot+L_cold)/2 token-columns regardless
    of routing skew, while keeping per-core weight traffic identical to
    one full expert (weights stream from HBM exactly once).
  - All matmul operands are bf16 (PE rate is identical to fp32r, but DMA
    bytes and SBUF footprint halve); PSUM accumulation is fp32 and the
    partial outputs return as fp32.
  - Host sums the two half partials, applies combine weights, and
    scatter-adds into token order.

Device layout: activations are transposed ([feature, token]); x lives in
SBUF as [128, 8, cap] (k-subtile middle), h as [128, 16, cap] bf16 which
fits residently, so phase 2 needs no hidden-dim quartering and y needs no
multi-pass accumulation: each phase-2 psum group covers the full 2048
contraction and evacuates straight to the output DMA.
"""

import numpy as np

N_EMBED = 1024
FFN_HIDDEN = 4096
NUM_EXPERTS = 8
TOP_K = 2
P = 128
KT1 = N_EMBED // P          # 8  k-tiles, phase 1
FH = FFN_HIDDEN // 2        # 2048 hidden features per core (half)
MT1 = FH // P               # 16 m-tiles, phase 1 (half hidden)
KT2 = FH // P               # 16 k-tiles, phase 2
MT2 = N_EMBED // P          # 8  m-tiles, phase 2

LAST_RESULT = None          # BassKernelResults of the most recent run


def _ensure_axon_hooks():
    """Make `antenv.axon_hooks` importable so BASS_TRACE=1 degrades
    gracefully instead of crashing when the image lacks the module."""
    try:
        import antenv.axon_hooks  # noqa: F401
        return
    except ImportError:
        pass
    import sys
    import types

    m = types.ModuleType("antenv.axon_hooks")
    m._hook = None
    m.set_axon_ntff_profile_hook = lambda h: setattr(m, "_hook", h)
    m.get_axon_ntff_profile_hook = lambda: m._hook
    sys.modules["antenv.axon_hooks"] = m
    try:
        from trn_agent_boot.trn_boot import _ntff_profile_via_ctypes

        m.set_axon_ntff_profile_hook(_ntff_profile_via_ctypes("/opt/axon/libaxon_pjrt.so"))
    except Exception:
        pass


def _route(x2d, Wg, bg):
    """Top-2 gating. Mirrors the reference (jax softmax + lax.top_k) so the
    selected experts match it exactly; numpy fallback is numerically
    equivalent up to fp32 rounding."""
    try:
        import jax
        import jax.numpy as jnp

        gate = jax.nn.softmax(jnp.asarray(x2d) @ jnp.asarray(Wg) + jnp.asarray(bg), axis=-1)
        scores, idx = jax.lax.top_k(gate, TOP_K)
        scores = np.asarray(scores, dtype=np.float32)
        idx = np.asarray(idx)
    except Exception:
        logits = x2d @ Wg + bg
        m = logits.max(-1, keepdims=True)
        e = np.exp(logits - m)
        p = e / e.sum(-1, keepdims=True)
        order = np.argsort(-p, axis=-1, kind="stable")
        idx = order[:, :TOP_K]
        scores = np.take_along_axis(p, idx, axis=-1)
    scores = scores / (scores.sum(-1, keepdims=True) + 1e-8)
    return idx.astype(np.int64), scores.astype(np.float32)


def _chunks(width, reverse=False):
    """Split a token capacity into matmul free-dim chunks of at most 512
    (one PSUM bank of fp32). Smallest chunk first so the kernel's very
    first psum group is short; reverse=True puts it last (for the tail)."""
    out, rem = [], width
    while rem > 0:
        c = min(rem, 512)
        out.append(c)
        rem -= c
    out.sort(reverse=reverse)
    res, off = [], 0
    for c in out:
        res.append((off, c))
        off += c
    return res


def _build_device_program(capA, capB):
    import concourse.tile as tile
    from concourse import bacc, mybir

    f32 = mybir.dt.float32
    bf16 = mybir.dt.bfloat16
    gelu = mybir.ActivationFunctionType.Gelu_apprx_tanh

    nc = bacc.Bacc("TRN2", target_bir_lowering=False, debug=False,
                   num_devices=NUM_EXPERTS)

    dram = {}
    for tag, cap in (("a", capA), ("b", capB)):
        dram[f"xg{tag}"] = nc.dram_tensor(f"xg{tag}", [P, KT1, cap], bf16,
                                          kind="ExternalInput").ap()
        dram[f"w1{tag}"] = nc.dram_tensor(f"w1{tag}", [MT1, P, KT1 * P], bf16,
                                          kind="ExternalInput").ap()
        dram[f"w2{tag}"] = nc.dram_tensor(f"w2{tag}", [MT2, P, KT2 * P], bf16,
                                          kind="ExternalInput").ap()
        dram[f"b1{tag}"] = nc.dram_tensor(f"b1{tag}", [P, MT1], f32,
                                          kind="ExternalInput").ap()
        dram[f"b2{tag}"] = nc.dram_tensor(f"b2{tag}", [P, MT2], f32,
                                          kind="ExternalInput").ap()
        dram[f"y{tag}"] = nc.dram_tensor(f"y{tag}", [MT2, P, cap], f32,
                                         kind="ExternalOutput").ap()

    with tile.TileContext(nc) as tc:
        with (
            tc.tile_pool(name="const", bufs=1) as const,
            tc.tile_pool(name="xp", bufs=1) as xp,
            tc.tile_pool(name="hp", bufs=1) as hp,
            tc.tile_pool(name="w1p", bufs=4) as w1p,
            tc.tile_pool(name="w2p", bufs=3) as w2p,
            tc.tile_pool(name="psp", bufs=8, space="PSUM") as psp,
            tc.tile_pool(name="op", bufs=6) as op,
        ):
            chA = _chunks(capA)
            chB = _chunks(capB)

            # ---- prologue DMAs (sync queue: x + biases; gpsimd: weights) ----
            xa = xp.tile([P, KT1, capA], bf16, name="xa")
            xb = xp.tile([P, KT1, capB], bf16, name="xb")
            c0 = chA[0][1]
            nc.sync.dma_start(xa[:, :, :c0], dram["xga"][:, :, :c0])
            b1s, b2s = {}, {}
            for tag in ("a", "b"):
                b1s[tag] = const.tile([P, MT1], f32, name=f"b1{tag}")
                nc.sync.dma_start(b1s[tag][:], dram[f"b1{tag}"][:, :])
                b2s[tag] = const.tile([P, MT2], f32, name=f"b2{tag}")
                nc.sync.dma_start(b2s[tag][:], dram[f"b2{tag}"][:, :])
            nc.sync.dma_start(xa[:, :, c0:], dram["xga"][:, :, c0:])
            nc.sync.dma_start(xb[:], dram["xgb"][:])

            for tag, cap, chunks, x_sb in (("a", capA, chA, xa),
                                           ("b", capB, chB, xb)):
                hT = hp.tile([P, MT1, cap], bf16, name=f"h{tag}")
                # ---- phase 1: hT = gelu(W1h.T @ xT + b1h), all 16 m-tiles
                for m in range(MT1):
                    w1m = w1p.tile([P, KT1, P], bf16, tag="w1")
                    nc.gpsimd.dma_start(
                        w1m[:], dram[f"w1{tag}"][m].rearrange("p (k q) -> p k q", k=KT1))
                    for ci, (cs, cw) in enumerate(chunks):
                        ps = psp.tile([P, cw], f32, tag="ps", name=f"ps{ci}")
                        for kt in range(KT1):
                            nc.tensor.matmul(
                                ps[:],
                                w1m[:, kt, :],
                                x_sb[:, kt, cs:cs + cw],
                                start=(kt == 0),
                                stop=(kt == KT1 - 1),
                            )
                        nc.scalar.activation(
                            hT[:, m, cs:cs + cw], ps[:], gelu,
                            bias=b1s[tag][:, m:m + 1],
                        )
                # ---- phase 2: y = W2h.T @ hT + b2 (partial; host sums halves)
                for m in range(MT2):
                    w2m = w2p.tile([P, KT2, P], bf16, tag="w2")
                    nc.gpsimd.dma_start(
                        w2m[:], dram[f"w2{tag}"][m].rearrange("p (k q) -> p k q", k=KT2))
                    mchunks = chunks
                    if tag == "b" and m == MT2 - 1:
                        mchunks = _chunks(cap, reverse=True)
                    for ci, (cs, cw) in enumerate(mchunks):
                        ps = psp.tile([P, cw], f32, tag="ps", name=f"ps{ci}")
                        for kq in range(KT2):
                            nc.tensor.matmul(
                                ps[:],
                                w2m[:, kq, :],
                                hT[:, kq, cs:cs + cw],
                                start=(kq == 0),
                                stop=(kq == KT2 - 1),
                            )
                        ot = op.tile([P, cw], f32, tag="o", name=f"o{ci}")
                        nc.vector.tensor_scalar_add(ot[:], ps[:], b2s[tag][:, m:m + 1])
                        nc.sync.dma_start(dram[f"y{tag}"][m, :, cs:cs + cw], ot[:])

    nc.compile()
    return nc


def _pad16(n):
    return max(256, -(-n // 16) * 16)


def kernel(x, Wg, bg, W1, b1, W2, b2):
    global LAST_RESULT
    _ensure_axon_hooks()
    import ml_dtypes
    from concourse.bass_utils import run_bass_kernel_spmd

    bf = ml_dtypes.bfloat16
    x = np.ascontiguousarray(np.asarray(x, dtype=np.float32))
    Wg = np.asarray(Wg, dtype=np.float32)
    bg = np.asarray(bg, dtype=np.float32)
    W1 = np.asarray(W1, dtype=np.float32)
    b1 = np.asarray(b1, dtype=np.float32)
    W2 = np.asarray(W2, dtype=np.float32)
    b2 = np.asarray(b2, dtype=np.float32)

    B, S, D = x.shape
    T = B * S
    xf = x.reshape(T, D)

    top_idx, top_w = _route(xf, Wg, bg)

    tok_idx, tok_w = [], []
    for e in range(NUM_EXPERTS):
        sel = top_idx == e
        rows = np.nonzero(sel.any(axis=1))[0]
        tok_idx.append(rows)
        tok_w.append((top_w * sel).sum(axis=1)[rows].astype(np.float32))

    loads = [len(r) for r in tok_idx]
    order = np.argsort(-np.asarray(loads), kind="stable")
    # pair hottest with coldest: pair p = (order[p], order[7-p])
    pairs = [(int(order[p]), int(order[NUM_EXPERTS - 1 - p]))
             for p in range(NUM_EXPERTS // 2)]
    capA = _pad16(max(loads[a] for a, _ in pairs))
    capB = _pad16(max(loads[b] for _, b in pairs))

    nc = _build_device_program(capA, capB)

    def prep_x(e, cap):
        idx_pad = np.zeros(cap, dtype=np.int64)
        idx_pad[:loads[e]] = tok_idx[e]
        xg = xf[idx_pad].T.reshape(KT1, P, cap).transpose(1, 0, 2)
        return np.ascontiguousarray(xg.astype(bf))

    def prep_w(e, half):
        fh = slice(half * FH, (half + 1) * FH)
        w1t = np.ascontiguousarray(
            W1[e][:, fh].reshape(KT1, P, MT1, P).transpose(2, 1, 0, 3)
            .reshape(MT1, P, KT1 * P).astype(bf))
        w2t = np.ascontiguousarray(
            W2[e][fh, :].reshape(KT2, P, MT2, P).transpose(2, 1, 0, 3)
            .reshape(MT2, P, KT2 * P).astype(bf))
        b1t = np.ascontiguousarray(b1[e][fh].reshape(MT1, P).T)
        if half == 0:
            b2t = np.ascontiguousarray(b2[e].reshape(MT2, P).T)
        else:
            b2t = np.zeros((P, MT2), dtype=np.float32)
        return w1t, w2t, b1t, b2t

    in_maps = []
    xg_cache = {}
    for p, (ea, eb) in enumerate(pairs):
        xg_cache[ea] = prep_x(ea, capA)
        xg_cache[eb] = prep_x(eb, capB)
        for half in range(2):
            w1a, w2a, b1a, b2a = prep_w(ea, half)
            w1b, w2b, b1b, b2b = prep_w(eb, half)
            in_maps.append({
                "xga": xg_cache[ea], "w1a": w1a, "w2a": w2a, "b1a": b1a, "b2a": b2a,
                "xgb": xg_cache[eb], "w1b": w1b, "w2b": w2b, "b1b": b1b, "b2b": b2b,
            })

    import os
    trace_cores = None
    if os.environ.get("MOE_TRACE_ALL"):
        trace_cores = list(range(NUM_EXPERTS))
    res = run_bass_kernel_spmd(nc, in_maps, core_ids=list(range(NUM_EXPERTS)),
                               trace_cores=trace_cores)
    LAST_RESULT = res

    out = np.zeros((T, D), dtype=np.float32)
    for p, (ea, eb) in enumerate(pairs):
        for e, key, cap in ((ea, "ya", capA), (eb, "yb", capB)):
            n_e = loads[e]
            if n_e == 0:
                continue
            yT = (res.results[2 * p][key].astype(np.float32)
                  + res.results[2 * p + 1][key].astype(np.float32)).reshape(D, cap)
            out[tok_idx[e]] += tok_w[e][:, None] * yT[:, :n_e].T
    return out.reshape(B, S, D)


# revision 9
# speedup vs baseline: 1.1174x; 1.0052x over previous
"""MoE layer (8 experts, top-2) on 8 Trainium2 NeuronCores.

Strategy (v2): hidden-dim-split expert parallelism in bf16.
  - Host computes gating + top-2 routing (mirrors the reference ops).
  - Experts are sorted by token load and paired hot-with-cold; pair p is
    assigned to cores (2p, 2p+1), each core computing one HALF of the FFN
    hidden dim (2048 of 4096) for BOTH experts of the pair. This balances
    the per-core matmul work to ~(L_hot+L_cold)/2 token-columns regardless
    of routing skew, while keeping per-core weight traffic identical to
    one full expert (weights stream from HBM exactly once).
  - All matmul operands are bf16 (PE rate is identical to fp32r, but DMA
    bytes and SBUF footprint halve); PSUM accumulation is fp32 and the
    partial outputs return as fp32.
  - Host sums the two half partials, applies combine weights, and
    scatter-adds into token order.

Device layout: activations are transposed ([feature, token]); x lives in
SBUF as [128, 8, cap] (k-subtile middle), h as [128, 16, cap] bf16 which
fits residently, so phase 2 needs no hidden-dim quartering and y needs no
multi-pass accumulation: each phase-2 psum group covers the full 2048
contraction and evacuates straight to the output DMA.
"""

import numpy as np

N_EMBED = 1024
FFN_HIDDEN = 4096
NUM_EXPERTS = 8
TOP_K = 2
P = 128
KT1 = N_EMBED // P          # 8  k-tiles, phase 1
FH = FFN_HIDDEN // 2        # 2048 hidden features per core (half)
MT1 = FH // P               # 16 m-tiles, phase 1 (half hidden)
KT2 = FH // P               # 16 k-tiles, phase 2
MT2 = N_EMBED // P          # 8  m-tiles, phase 2

LAST_RESULT = None          # BassKernelResults of the most recent run


def _ensure_axon_hooks():
    """Make `antenv.axon_hooks` importable so BASS_TRACE=1 degrades
    gracefully instead of crashing when the image lacks the module."""
    try:
        import antenv.axon_hooks  # noqa: F401
        return
    except ImportError:
        pass
    import sys
    import types

    m = types.ModuleType("antenv.axon_hooks")
    m._hook = None
    m.set_axon_ntff_profile_hook = lambda h: setattr(m, "_hook", h)
    m.get_axon_ntff_profile_hook = lambda: m._hook
    sys.modules["antenv.axon_hooks"] = m
    try:
        from trn_agent_boot.trn_boot import _ntff_profile_via_ctypes

        m.set_axon_ntff_profile_hook(_ntff_profile_via_ctypes("/opt/axon/libaxon_pjrt.so"))
    except Exception:
        pass


def _route(x2d, Wg, bg):
    """Top-2 gating. Mirrors the reference (jax softmax + lax.top_k) so the
    selected experts match it exactly; numpy fallback is numerically
    equivalent up to fp32 rounding."""
    try:
        import jax
        import jax.numpy as jnp

        gate = jax.nn.softmax(jnp.asarray(x2d) @ jnp.asarray(Wg) + jnp.asarray(bg), axis=-1)
        scores, idx = jax.lax.top_k(gate, TOP_K)
        scores = np.asarray(scores, dtype=np.float32)
        idx = np.asarray(idx)
    except Exception:
        logits = x2d @ Wg + bg
        m = logits.max(-1, keepdims=True)
        e = np.exp(logits - m)
        p = e / e.sum(-1, keepdims=True)
        order = np.argsort(-p, axis=-1, kind="stable")
        idx = order[:, :TOP_K]
        scores = np.take_along_axis(p, idx, axis=-1)
    scores = scores / (scores.sum(-1, keepdims=True) + 1e-8)
    return idx.astype(np.int64), scores.astype(np.float32)


def _chunks(width, reverse=False):
    """Split a token capacity into matmul free-dim chunks of at most 512
    (one PSUM bank of fp32). Smallest chunk first so the kernel's very
    first psum group is short; reverse=True puts it last (for the tail)."""
    out, rem = [], width
    while rem > 0:
        c = min(rem, 512)
        out.append(c)
        rem -= c
    out.sort(reverse=reverse)
    res, off = [], 0
    for c in out:
        res.append((off, c))
        off += c
    return res


def _build_device_program(capA, capB):
    import concourse.tile as tile
    from concourse import bacc, mybir

    f32 = mybir.dt.float32
    bf16 = mybir.dt.bfloat16
    gelu = mybir.ActivationFunctionType.Gelu_apprx_tanh

    nc = bacc.Bacc("TRN2", target_bir_lowering=False, debug=False,
                   num_devices=NUM_EXPERTS)

    dram = {}
    for tag, cap in (("a", capA), ("b", capB)):
        dram[f"xg{tag}"] = nc.dram_tensor(f"xg{tag}", [P, KT1, cap], bf16,
                                          kind="ExternalInput").ap()
        dram[f"w1{tag}"] = nc.dram_tensor(f"w1{tag}", [MT1, P, KT1 * P], bf16,
                                          kind="ExternalInput").ap()
        dram[f"w2{tag}"] = nc.dram_tensor(f"w2{tag}", [MT2, P, KT2 * P], bf16,
                                          kind="ExternalInput").ap()
        dram[f"b1{tag}"] = nc.dram_tensor(f"b1{tag}", [P, MT1], f32,
                                          kind="ExternalInput").ap()
        dram[f"y{tag}"] = nc.dram_tensor(f"y{tag}", [MT2, P, cap], f32,
                                         kind="ExternalOutput").ap()

    with tile.TileContext(nc) as tc:
        with (
            tc.tile_pool(name="const", bufs=1) as const,
            tc.tile_pool(name="xp", bufs=1) as xp,
            tc.tile_pool(name="hp", bufs=1) as hp,
            tc.tile_pool(name="w1p", bufs=4) as w1p,
            tc.tile_pool(name="w2p", bufs=3) as w2p,
            tc.tile_pool(name="psp", bufs=8, space="PSUM") as psp,
            tc.tile_pool(name="op", bufs=6) as op,
        ):
            chA = _chunks(capA)
            chB = _chunks(capB)

            # ---- prologue DMAs (sync queue: x + biases; gpsimd: weights) ----
            xa = xp.tile([P, KT1, capA], bf16, name="xa")
            xb = xp.tile([P, KT1, capB], bf16, name="xb")
            c0 = chA[0][1]
            nc.sync.dma_start(xa[:, :, :c0], dram["xga"][:, :, :c0])
            b1s = {}
            for tag in ("a", "b"):
                b1s[tag] = const.tile([P, MT1], f32, name=f"b1{tag}")
                nc.sync.dma_start(b1s[tag][:], dram[f"b1{tag}"][:, :])
            nc.sync.dma_start(xa[:, :, c0:], dram["xga"][:, :, c0:])
            nc.sync.dma_start(xb[:], dram["xgb"][:])

            # PE warm-up: the tensor engine clock ramps (0.65 -> 1.2 ->
            # 2.4 GHz over ~3us of sustained activity). Dummy matmuls on a
            # zero tile keep it busy while x streams in, so the real work
            # starts at full clock.
            warm = const.tile([P, 256], bf16, name="warm")
            nc.vector.memset(warm[:], 0)
            wps = psp.tile([P, 256], f32, tag="ps", name="warmps")
            for _ in range(16):
                nc.tensor.matmul(wps[:], warm[:, :128], warm[:],
                                 start=True, stop=True)

            for tag, cap, chunks, x_sb in (("a", capA, chA, xa),
                                           ("b", capB, chB, xb)):
                hT = hp.tile([P, MT1, cap], bf16, name=f"h{tag}")
                # ---- phase 1: hT = gelu(W1h.T @ xT + b1h), all 16 m-tiles
                for m in range(MT1):
                    w1m = w1p.tile([P, KT1, P], bf16, tag="w1")
                    nc.gpsimd.dma_start(
                        w1m[:], dram[f"w1{tag}"][m].rearrange("p (k q) -> p k q", k=KT1))
                    for ci, (cs, cw) in enumerate(chunks):
                        ps = psp.tile([P, cw], f32, tag="ps", name=f"ps{ci}")
                        for kt in range(KT1):
                            nc.tensor.matmul(
                                ps[:],
                                w1m[:, kt, :],
                                x_sb[:, kt, cs:cs + cw],
                                start=(kt == 0),
                                stop=(kt == KT1 - 1),
                            )
                        nc.scalar.activation(
                            hT[:, m, cs:cs + cw], ps[:], gelu,
                            bias=b1s[tag][:, m:m + 1],
                        )
                # ---- phase 2: y = W2h.T @ hT + b2 (partial; host sums halves)
                for m in range(MT2):
                    w2m = w2p.tile([P, KT2, P], bf16, tag="w2")
                    nc.gpsimd.dma_start(
                        w2m[:], dram[f"w2{tag}"][m].rearrange("p (k q) -> p k q", k=KT2))
                    mchunks = chunks
                    if tag == "b" and m == MT2 - 1:
                        mchunks = _chunks(cap, reverse=True)
                    for ci, (cs, cw) in enumerate(mchunks):
                        ps = psp.tile([P, cw], f32, tag="ps", name=f"ps{ci}")
                        for kq in range(KT2):
                            nc.tensor.matmul(
                                ps[:],
                                w2m[:, kq, :],
                                hT[:, kq, cs:cs + cw],
                                start=(kq == 0),
                                stop=(kq == KT2 - 1),
                            )
                        # evacuate psum and store; b2 is added on the host
                        # during the combine step
                        ot = op.tile([P, cw], f32, tag="o", name=f"o{ci}")
                        nc.vector.tensor_copy(ot[:], ps[:])
                        nc.sync.dma_start(dram[f"y{tag}"][m, :, cs:cs + cw], ot[:])

    nc.compile()
    return nc


def _pad16(n):
    return max(256, -(-n // 16) * 16)


def kernel(x, Wg, bg, W1, b1, W2, b2):
    global LAST_RESULT
    _ensure_axon_hooks()
    import ml_dtypes
    from concourse.bass_utils import run_bass_kernel_spmd

    bf = ml_dtypes.bfloat16
    x = np.ascontiguousarray(np.asarray(x, dtype=np.float32))
    Wg = np.asarray(Wg, dtype=np.float32)
    bg = np.asarray(bg, dtype=np.float32)
    W1 = np.asarray(W1, dtype=np.float32)
    b1 = np.asarray(b1, dtype=np.float32)
    W2 = np.asarray(W2, dtype=np.float32)
    b2 = np.asarray(b2, dtype=np.float32)

    B, S, D = x.shape
    T = B * S
    xf = x.reshape(T, D)

    top_idx, top_w = _route(xf, Wg, bg)

    tok_idx, tok_w = [], []
    for e in range(NUM_EXPERTS):
        sel = top_idx == e
        rows = np.nonzero(sel.any(axis=1))[0]
        tok_idx.append(rows)
        tok_w.append((top_w * sel).sum(axis=1)[rows].astype(np.float32))

    loads = [len(r) for r in tok_idx]
    order = np.argsort(-np.asarray(loads), kind="stable")
    # pair hottest with coldest: pair p = (order[p], order[7-p])
    pairs = [(int(order[p]), int(order[NUM_EXPERTS - 1 - p]))
             for p in range(NUM_EXPERTS // 2)]
    capA = _pad16(max(loads[a] for a, _ in pairs))
    capB = _pad16(max(loads[b] for _, b in pairs))

    nc = _build_device_program(capA, capB)

    def prep_x(e, cap):
        idx_pad = np.zeros(cap, dtype=np.int64)
        idx_pad[:loads[e]] = tok_idx[e]
        xg = xf[idx_pad].T.reshape(KT1, P, cap).transpose(1, 0, 2)
        return np.ascontiguousarray(xg.astype(bf))

    def prep_w(e, half):
        fh = slice(half * FH, (half + 1) * FH)
        w1t = np.ascontiguousarray(
            W1[e][:, fh].reshape(KT1, P, MT1, P).transpose(2, 1, 0, 3)
            .reshape(MT1, P, KT1 * P).astype(bf))
        w2t = np.ascontiguousarray(
            W2[e][fh, :].reshape(KT2, P, MT2, P).transpose(2, 1, 0, 3)
            .reshape(MT2, P, KT2 * P).astype(bf))
        b1t = np.ascontiguousarray(b1[e][fh].reshape(MT1, P).T)
        return w1t, w2t, b1t

    in_maps = []
    xg_cache = {}
    for p, (ea, eb) in enumerate(pairs):
        xg_cache[ea] = prep_x(ea, capA)
        xg_cache[eb] = prep_x(eb, capB)
        for half in range(2):
            w1a, w2a, b1a = prep_w(ea, half)
            w1b, w2b, b1b = prep_w(eb, half)
            in_maps.append({
                "xga": xg_cache[ea], "w1a": w1a, "w2a": w2a, "b1a": b1a,
                "xgb": xg_cache[eb], "w1b": w1b, "w2b": w2b, "b1b": b1b,
            })

    import os
    trace_cores = None
    if os.environ.get("MOE_TRACE_ALL"):
        trace_cores = list(range(NUM_EXPERTS))
    res = run_bass_kernel_spmd(nc, in_maps, core_ids=list(range(NUM_EXPERTS)),
                               trace_cores=trace_cores)
    LAST_RESULT = res

    out = np.zeros((T, D), dtype=np.float32)
    for p, (ea, eb) in enumerate(pairs):
        for e, key, cap in ((ea, "ya", capA), (eb, "yb", capB)):
            n_e = loads[e]
            if n_e == 0:
                continue
            yT = (res.results[2 * p][key].astype(np.float32)
                  + res.results[2 * p + 1][key].astype(np.float32)).reshape(D, cap)
            out[tok_idx[e]] += tok_w[e][:, None] * (yT[:, :n_e].T + b2[e])
    return out.reshape(B, S, D)


# revision 13
# speedup vs baseline: 1.1240x; 1.0058x over previous
"""MoE layer (8 experts, top-2) on 8 Trainium2 NeuronCores.

Strategy (v2): hidden-dim-split expert parallelism in bf16.
  - Host computes gating + top-2 routing (mirrors the reference ops).
  - Experts are sorted by token load and paired hot-with-cold; pair p is
    assigned to cores (2p, 2p+1), each core computing one HALF of the FFN
    hidden dim (2048 of 4096) for BOTH experts of the pair. This balances
    the per-core matmul work to ~(L_hot+L_cold)/2 token-columns regardless
    of routing skew, while keeping per-core weight traffic identical to
    one full expert (weights stream from HBM exactly once).
  - All matmul operands are bf16 (PE rate is identical to fp32r, but DMA
    bytes and SBUF footprint halve); PSUM accumulation is fp32 and the
    partial outputs return as fp32.
  - Host sums the two half partials, applies combine weights, and
    scatter-adds into token order.

Device layout: activations are transposed ([feature, token]); x lives in
SBUF as [128, 8, cap] (k-subtile middle), h as [128, 16, cap] bf16 which
fits residently, so phase 2 needs no hidden-dim quartering and y needs no
multi-pass accumulation: each phase-2 psum group covers the full 2048
contraction and evacuates straight to the output DMA.
"""

import numpy as np

N_EMBED = 1024
FFN_HIDDEN = 4096
NUM_EXPERTS = 8
TOP_K = 2
P = 128
KT1 = N_EMBED // P          # 8  k-tiles, phase 1
FH = FFN_HIDDEN // 2        # 2048 hidden features per core (half)
MT1 = FH // P               # 16 m-tiles, phase 1 (half hidden)
KT2 = FH // P               # 16 k-tiles, phase 2
MT2 = N_EMBED // P          # 8  m-tiles, phase 2

LAST_RESULT = None          # BassKernelResults of the most recent run


def _ensure_axon_hooks():
    """Make `antenv.axon_hooks` importable so BASS_TRACE=1 degrades
    gracefully instead of crashing when the image lacks the module."""
    try:
        import antenv.axon_hooks  # noqa: F401
        return
    except ImportError:
        pass
    import sys
    import types

    m = types.ModuleType("antenv.axon_hooks")
    m._hook = None
    m.set_axon_ntff_profile_hook = lambda h: setattr(m, "_hook", h)
    m.get_axon_ntff_profile_hook = lambda: m._hook
    sys.modules["antenv.axon_hooks"] = m
    try:
        from trn_agent_boot.trn_boot import _ntff_profile_via_ctypes

        m.set_axon_ntff_profile_hook(_ntff_profile_via_ctypes("/opt/axon/libaxon_pjrt.so"))
    except Exception:
        pass


def _route(x2d, Wg, bg):
    """Top-2 gating. Mirrors the reference (jax softmax + lax.top_k) so the
    selected experts match it exactly; numpy fallback is numerically
    equivalent up to fp32 rounding."""
    try:
        import jax
        import jax.numpy as jnp

        gate = jax.nn.softmax(jnp.asarray(x2d) @ jnp.asarray(Wg) + jnp.asarray(bg), axis=-1)
        scores, idx = jax.lax.top_k(gate, TOP_K)
        scores = np.asarray(scores, dtype=np.float32)
        idx = np.asarray(idx)
    except Exception:
        logits = x2d @ Wg + bg
        m = logits.max(-1, keepdims=True)
        e = np.exp(logits - m)
        p = e / e.sum(-1, keepdims=True)
        order = np.argsort(-p, axis=-1, kind="stable")
        idx = order[:, :TOP_K]
        scores = np.take_along_axis(p, idx, axis=-1)
    scores = scores / (scores.sum(-1, keepdims=True) + 1e-8)
    return idx.astype(np.int64), scores.astype(np.float32)


def _chunks(width, reverse=False):
    """Split a token capacity into matmul free-dim chunks of at most 512
    (one PSUM bank of fp32). Smallest chunk first so the kernel's very
    first psum group is short; reverse=True puts it last (for the tail)."""
    out, rem = [], width
    while rem > 0:
        c = min(rem, 512)
        out.append(c)
        rem -= c
    out.sort(reverse=reverse)
    res, off = [], 0
    for c in out:
        res.append((off, c))
        off += c
    return res


def _build_device_program(capA, capB):
    import concourse.tile as tile
    from concourse import bacc, mybir

    f32 = mybir.dt.float32
    bf16 = mybir.dt.bfloat16
    gelu = mybir.ActivationFunctionType.Gelu_apprx_tanh

    nc = bacc.Bacc("TRN2", target_bir_lowering=False, debug=False,
                   num_devices=NUM_EXPERTS)

    dram = {}
    for tag, cap in (("a", capA), ("b", capB)):
        # x is stored chunk-major: contiguous [P, KT1*cw] blocks per chunk,
        # so every chunk DMA moves contiguous per-partition runs at full
        # bandwidth (a strided whole-tensor DMA measured ~60% slower and
        # stalled the first phase for ~12us).
        dram[f"xg{tag}"] = nc.dram_tensor(f"xg{tag}", [P, KT1 * cap], bf16,
                                          kind="ExternalInput").ap()
        dram[f"w1{tag}"] = nc.dram_tensor(f"w1{tag}", [MT1, P, KT1 * P], bf16,
                                          kind="ExternalInput").ap()
        dram[f"w2{tag}"] = nc.dram_tensor(f"w2{tag}", [MT2, P, KT2 * P], bf16,
                                          kind="ExternalInput").ap()
        dram[f"b1{tag}"] = nc.dram_tensor(f"b1{tag}", [P, MT1], f32,
                                          kind="ExternalInput").ap()
        dram[f"y{tag}"] = nc.dram_tensor(f"y{tag}", [MT2, P, cap], f32,
                                         kind="ExternalOutput").ap()

    with tile.TileContext(nc) as tc:
        with (
            tc.tile_pool(name="const", bufs=1) as const,
            tc.tile_pool(name="xp", bufs=1) as xp,
            tc.tile_pool(name="hp", bufs=1) as hp,
            tc.tile_pool(name="w1p", bufs=4) as w1p,
            tc.tile_pool(name="w2p", bufs=3) as w2p,
            tc.tile_pool(name="psp", bufs=8, space="PSUM") as psp,
            tc.tile_pool(name="op", bufs=6) as op,
        ):
            chA = _chunks(capA)
            chB = _chunks(capB)

            # ---- prologue DMAs (sync queue: x + biases; gpsimd: weights) ----
            xa = xp.tile([P, KT1, capA], bf16, name="xa")
            xb = xp.tile([P, KT1, capB], bf16, name="xb")
            b1s = {}
            for tag in ("a", "b"):
                b1s[tag] = const.tile([P, MT1], f32, name=f"b1{tag}")
                nc.sync.dma_start(b1s[tag][:], dram[f"b1{tag}"][:, :])
            for tag, chunks, x_sb in (("a", chA, xa), ("b", chB, xb)):
                for (cs, cw) in chunks:
                    nc.sync.dma_start(
                        x_sb[:, :, cs:cs + cw],
                        dram[f"xg{tag}"][:, 8 * cs:8 * (cs + cw)]
                        .rearrange("p (k c) -> p k c", k=KT1))

            for tag, cap, chunks, x_sb in (("a", capA, chA, xa),
                                           ("b", capB, chB, xb)):
                hT = hp.tile([P, MT1, cap], bf16, name=f"h{tag}")
                # ---- phase 1: hT = gelu(W1h.T @ xT + b1h), all 16 m-tiles
                for m in range(MT1):
                    w1m = w1p.tile([P, KT1, P], bf16, tag="w1")
                    nc.gpsimd.dma_start(
                        w1m[:], dram[f"w1{tag}"][m].rearrange("p (k q) -> p k q", k=KT1))
                    for ci, (cs, cw) in enumerate(chunks):
                        ps = psp.tile([P, cw], f32, tag="ps", name=f"ps{ci}")
                        for kt in range(KT1):
                            nc.tensor.matmul(
                                ps[:],
                                w1m[:, kt, :],
                                x_sb[:, kt, cs:cs + cw],
                                start=(kt == 0),
                                stop=(kt == KT1 - 1),
                            )
                        nc.scalar.activation(
                            hT[:, m, cs:cs + cw], ps[:], gelu,
                            bias=b1s[tag][:, m:m + 1],
                        )
                # ---- phase 2: y = W2h.T @ hT + b2 (partial; host sums halves)
                for m in range(MT2):
                    w2m = w2p.tile([P, KT2, P], bf16, tag="w2")
                    nc.gpsimd.dma_start(
                        w2m[:], dram[f"w2{tag}"][m].rearrange("p (k q) -> p k q", k=KT2))
                    mchunks = chunks
                    if tag == "b" and m == MT2 - 1:
                        mchunks = _chunks(cap, reverse=True)
                    for ci, (cs, cw) in enumerate(mchunks):
                        ps = psp.tile([P, cw], f32, tag="ps", name=f"ps{ci}")
                        for kq in range(KT2):
                            nc.tensor.matmul(
                                ps[:],
                                w2m[:, kq, :],
                                hT[:, kq, cs:cs + cw],
                                start=(kq == 0),
                                stop=(kq == KT2 - 1),
                            )
                        # evacuate psum and store; b2 is added on the host
                        # during the combine step. The final phase's stores
                        # alternate between the sync and gpsimd DMA queues so
                        # the kernel tail isn't one serialized store queue
                        # (the gpsimd queue is done with weights by then).
                        ot = op.tile([P, cw], f32, tag="o", name=f"o{ci}")
                        nc.vector.tensor_copy(ot[:], ps[:])
                        q = nc.sync
                        if tag == "b" and m % 2 == 0:
                            q = nc.gpsimd
                        q.dma_start(dram[f"y{tag}"][m, :, cs:cs + cw], ot[:])

    nc.compile()
    return nc


def _pad16(n):
    return max(256, -(-n // 16) * 16)


def kernel(x, Wg, bg, W1, b1, W2, b2):
    global LAST_RESULT
    _ensure_axon_hooks()
    import ml_dtypes
    from concourse.bass_utils import run_bass_kernel_spmd

    bf = ml_dtypes.bfloat16
    x = np.ascontiguousarray(np.asarray(x, dtype=np.float32))
    Wg = np.asarray(Wg, dtype=np.float32)
    bg = np.asarray(bg, dtype=np.float32)
    W1 = np.asarray(W1, dtype=np.float32)
    b1 = np.asarray(b1, dtype=np.float32)
    W2 = np.asarray(W2, dtype=np.float32)
    b2 = np.asarray(b2, dtype=np.float32)

    B, S, D = x.shape
    T = B * S
    xf = x.reshape(T, D)

    top_idx, top_w = _route(xf, Wg, bg)

    tok_idx, tok_w = [], []
    for e in range(NUM_EXPERTS):
        sel = top_idx == e
        rows = np.nonzero(sel.any(axis=1))[0]
        tok_idx.append(rows)
        tok_w.append((top_w * sel).sum(axis=1)[rows].astype(np.float32))

    loads = [len(r) for r in tok_idx]
    order = np.argsort(-np.asarray(loads), kind="stable")
    # pair hottest with coldest: pair p = (order[p], order[7-p])
    pairs = [(int(order[p]), int(order[NUM_EXPERTS - 1 - p]))
             for p in range(NUM_EXPERTS // 2)]
    capA = _pad16(max(loads[a] for a, _ in pairs))
    capB = _pad16(max(loads[b] for _, b in pairs))

    nc = _build_device_program(capA, capB)

    def prep_x(e, cap):
        idx_pad = np.zeros(cap, dtype=np.int64)
        idx_pad[:loads[e]] = tok_idx[e]
        xg = xf[idx_pad].T.reshape(KT1, P, cap).transpose(1, 0, 2).astype(bf)
        # chunk-major: one contiguous [P, KT1*cw] block per chunk
        blocks = [np.ascontiguousarray(xg[:, :, cs:cs + cw]).reshape(P, -1)
                  for cs, cw in _chunks(cap)]
        return np.ascontiguousarray(np.concatenate(blocks, axis=1))

    def prep_w(e, half):
        fh = slice(half * FH, (half + 1) * FH)
        w1t = np.ascontiguousarray(
            W1[e][:, fh].reshape(KT1, P, MT1, P).transpose(2, 1, 0, 3)
            .reshape(MT1, P, KT1 * P).astype(bf))
        w2t = np.ascontiguousarray(
            W2[e][fh, :].reshape(KT2, P, MT2, P).transpose(2, 1, 0, 3)
            .reshape(MT2, P, KT2 * P).astype(bf))
        b1t = np.ascontiguousarray(b1[e][fh].reshape(MT1, P).T)
        return w1t, w2t, b1t

    in_maps = []
    xg_cache = {}
    for p, (ea, eb) in enumerate(pairs):
        xg_cache[ea] = prep_x(ea, capA)
        xg_cache[eb] = prep_x(eb, capB)
        for half in range(2):
            w1a, w2a, b1a = prep_w(ea, half)
            w1b, w2b, b1b = prep_w(eb, half)
            in_maps.append({
                "xga": xg_cache[ea], "w1a": w1a, "w2a": w2a, "b1a": b1a,
                "xgb": xg_cache[eb], "w1b": w1b, "w2b": w2b, "b1b": b1b,
            })

    import os
    trace_cores = None
    if os.environ.get("MOE_TRACE_ALL"):
        trace_cores = list(range(NUM_EXPERTS))
    res = run_bass_kernel_spmd(nc, in_maps, core_ids=list(range(NUM_EXPERTS)),
                               trace_cores=trace_cores)
    LAST_RESULT = res

    out = np.zeros((T, D), dtype=np.float32)
    for p, (ea, eb) in enumerate(pairs):
        for e, key, cap in ((ea, "ya", capA), (eb, "yb", capB)):
            n_e = loads[e]
            if n_e == 0:
                continue
            yT = (res.results[2 * p][key].astype(np.float32)
                  + res.results[2 * p + 1][key].astype(np.float32)).reshape(D, cap)
            out[tok_idx[e]] += tok_w[e][:, None] * (yT[:, :n_e].T + b2[e])
    return out.reshape(B, S, D)
